# revision 32
# baseline (speedup 1.0000x reference)
"""Trainium2 Bass kernel for AttentionBlock (B=8, C=256, L=2048), data-parallel
over batch across 8 NeuronCores.

Math (one batch per core, x: [C, L]):
    t^T   = w8^T x8            w8 = fp8(kappa M x),  M = Wq^T Wk,  kappa = 128*SCALE/ln2
    pT    = exp(t*ln2/128 + ux - shift)   [m, l], m on partitions; the global
                               shift cancels in softmax
    denom = ones^T acc(pT)     (running bf16 accumulator on DVE)
    ctx   = vT^T pT            vT = x^T Wv^T in bf16; ux (per-key bq.Wk x)
                               rides along as a 257th output column of the
                               same projection
    out   = ctx * (1/denom) + (bf16(x) + bv)

The C=256 contractions (w projection, v projection, scores) run in fp8e4 with
perf_mode=DoubleRow: operands packed [128, 2, free], one instruction contracts
256 deep. On this silicon DoubleRow matches bf16 ALU throughput, so its win is
instruction/LDWEIGHTS count, and pT/vT stay bf16 (fp8 elementwise ops on DVE
run at 1x and dominate otherwise; measured).

Schedule:
  - fp32 x is never loaded; the residual uses bf16 x and the output is stored
    bf16 (error budget allows it: rel_err ~3.8e-3 vs the 2e-2 gate)
  - the v projection rides inside the scores loop (one DoubleRow matmul per
    key chunk) sharing the PSUM pool with the score tiles, so there is no
    pool-transition barrier before the scores start
  - context accumulation for the left half of the queries (qt 0,1) is
    interleaved into the scores phase chunk by chunk (PSUM: 4 banks scores/vp
    + 4 banks ctx-left); the right half runs after from the stored pT
  - exp runs on ACT (4 x 512-wide slices per chunk, ~2.7us) pacing the PE
    (~2.7us/chunk); the denominator accumulates on DVE in bf16 (2x mode)
  - the residual prep is pinned behind the denominator matmul via a dummy
    data dependency so the scheduler cannot hoist it into the scores-phase
    DVE stream (DVE executes strictly in order; one early op delays every
    later consumer)
  - ACT/DVE table loads and PE warmup happen during the initial DMA
"""

import math
import numpy as np
import ml_dtypes

import concourse.bass as bass
import concourse.tile as tile
from concourse import bacc, mybir
from concourse.bass_utils import run_bass_kernel_spmd

B, C, L = 8, 256, 2048
P = 128                 # partitions
NMC = L // P            # 16 m-chunks (key blocks)
NPAIR = NMC // 2
NB = 512                # matmul moving free dim
HALF = 1024
SCALE = float(C) ** -0.5
LN2 = math.log(2.0)
KAPPA = 128.0 * SCALE / LN2     # scores t = kappa * s_raw (baked into mt8 on host)
SHIFT = 2.0                     # global exp shift; cancels in softmax
WARMUP_MMS = 8

F32 = mybir.dt.float32
BF16 = mybir.dt.bfloat16
F8 = mybir.dt.float8e4
DR = mybir.MatmulPerfMode.DoubleRow

_COMPILED = None


def build_nc():
    nc = bacc.Bacc("TRN2", target_bir_lowering=False, debug=False, num_devices=8)

    x8_d = nc.dram_tensor("x8", [C, L], F8, kind="ExternalInput").ap()
    xbf_d = nc.dram_tensor("xbf", [C, L], BF16, kind="ExternalInput").ap()
    mt8_d = nc.dram_tensor("mt8", [C, C], F8, kind="ExternalInput").ap()
    wvu8_d = nc.dram_tensor("wvu8", [C, 272], F8, kind="ExternalInput").ap()
    bv_d = nc.dram_tensor("bv", [C, 1], F32, kind="ExternalInput").ap()
    out_d = nc.dram_tensor("out", [C, L], BF16, kind="ExternalOutput").ap()

    with tile.TileContext(nc) as tc:
        with (
            tc.tile_pool(name="const", bufs=1) as const,
            tc.tile_pool(name="data", bufs=1) as data,
            tc.tile_pool(name="evict", bufs=4) as evict,
        ):
            # ---- constants / warmup fodder ----
            ones_bf = const.tile([P, NB], BF16)
            nc.vector.memset(ones_bf[:], 1.0)
            ones8 = const.tile([P, 2, 16], F8)
            nc.gpsimd.memset(ones8[:], 1.0)
            tiny = const.tile([P, 2, 16], F32)

            x8 = data.tile([P, 2, L], F8, tag="x8", name="x8")
            xbf = [data.tile([P, L], BF16, tag=f"xbf{c}", name=f"xbf{c}")
                   for c in range(2)]
            mt8 = const.tile([P, 2, C], F8, tag="mt8")
            wvu8 = const.tile([P, 2, 272], F8, tag="wvu8")
            bv_sb = const.tile([P, 2, 1], F32, tag="bv")

            # first l-slice of x8 on several queues, then weights, then rest
            def x8_dma(c0, c1, eng):
                cols = slice(c0, c1)
                eng.dma_start(out=x8[:, :, cols],
                              in_=x8_d[:, cols].rearrange("(j p) l -> p j l",
                                                          p=P))

            # mt8 first (the w projection is the startup critical path),
            # then x8 slices in consumption order across all three queues
            nc.sync.dma_start(out=mt8[:],
                              in_=mt8_d.rearrange("(j p) o -> p j o", p=P))
            x8_dma(512, 1024, nc.gpsimd)
            x8_dma(1024, 1536, nc.scalar)
            x8_dma(0, 512, nc.sync)
            x8_dma(1536, 2048, nc.gpsimd)
            nc.scalar.dma_start(out=wvu8[:],
                                in_=wvu8_d.rearrange("(j p) o -> p j o", p=P))
            nc.scalar.dma_start(out=bv_sb[:],
                                in_=bv_d.rearrange("(j p) o -> p j o", p=P))

            w8 = data.tile([P, 2, L], F8, tag="w8", name="w8")
            vT_bf = data.tile([P, NMC, C], BF16, tag="vT")
            pT_bf = data.tile([P, NMC, L], BF16, tag="pT")
            b_act = data.tile([P, NMC, 1], F32, tag="b_act")
            bv_late = data.tile([P, 2, 1], F32, tag="bv_late")
            dacc = data.tile([P, L], BF16, tag="dacc")
            recip = data.tile([P, L], F32, tag="recip")
            xr = [data.tile([P, L], BF16, tag=f"xr{c}", name=f"xr{c}")
                  for c in range(2)]

            # ---- phase 1: PE warmup + w projection ----
            with tc.tile_pool(name="psA", bufs=1, space=bass.MemorySpace.PSUM) as psA:
                # warm the activation tables (one-time ~2.7us DMAs) and the PE
                # HAM clock-gate while x streams in
                warm = psA.tile([P, HALF], F32, tag="wp", name="warm", bufs=2)
                nc.vector.memset(tiny[:, 0, :], 1.0)
                nc.scalar.activation(out=tiny[:, 1, :], in_=tiny[:, 0, :],
                                     func=mybir.ActivationFunctionType.Exp,
                                     scale=1.0)
                nc.vector.reciprocal_approx_fast(out=tiny[:, 1, :],
                                                 in_=tiny[:, 0, :])
                for i in range(WARMUP_MMS):
                    nc.tensor.matmul(warm[:, 0:NB], ones_bf[:, 0:P],
                                     ones_bf[:], start=True, stop=True)
                nc.tensor.matmul(warm[0:16, 0:16], ones8[:], ones8[:],
                                 start=True, stop=True, perf_mode=DR)

                # w = kappa M x  (kappa baked into mt8 on host); one DoubleRow
                # matmul contracts the full 256 channels
                for h in range(2):
                    hcols = slice(h * HALF, (h + 1) * HALF)
                    for oc in range(2):
                        wp = psA.tile([P, HALF], F32, tag="wp", name="wp",
                                      bufs=2)
                        for ln in range(2):
                            c0 = h * HALF + ln * NB
                            nc.tensor.matmul(
                                wp[:, ln * NB:(ln + 1) * NB],
                                mt8[:, :, oc * P:(oc + 1) * P],
                                x8[:, :, c0:c0 + NB],
                                start=True, stop=True, perf_mode=DR)
                        nc.vector.tensor_copy(out=w8[:, oc, hcols], in_=wp[:])

            # xbf for the residual - only needed by the epilogue; these queues
            # are idle during the scores phase
            nc.sync.dma_start(out=xbf[0][:], in_=xbf_d[0:P, :])
            nc.gpsimd.dma_start(out=xbf[1][:], in_=xbf_d[P:C, :])

            # ---- phase 2: v-projection + scores + exp + denom + ctx-left ----
            with tc.tile_pool(name="psCL", bufs=1,
                              space=bass.MemorySpace.PSUM) as psCL:
                ctxL = {(qt, cc): psCL.tile([P, NB], F32, tag=f"cl{qt}{cc}",
                                            name=f"cl{qt}{cc}", bufs=1)
                        for qt in range(2) for cc in range(2)}

                with tc.tile_pool(name="psB", bufs=1,
                                  space=bass.MemorySpace.PSUM) as psB:
                    for mc in range(NMC):
                        mrows = slice(mc * P, (mc + 1) * P)
                        # v/ux projection for this key chunk
                        vp = psB.tile([P, 272], F32, tag="vp", name="vp", bufs=1)
                        nc.tensor.matmul(
                            vp[:], x8[:, :, mrows], wvu8[:],
                            start=True, stop=True, perf_mode=DR)
                        nc.vector.tensor_copy(out=vT_bf[:, mc, :],
                                              in_=vp[:, 0:C])
                        nc.vector.tensor_scalar_add(out=b_act[:, mc, :],
                                                    in0=vp[:, C:C + 1],
                                                    scalar1=-SHIFT)
                        # scores + exp, 512 columns at a time
                        for qt in range(4):
                            s = psB.tile([P, NB], F32, tag="s", name="s",
                                         bufs=3)
                            nc.tensor.matmul(
                                s[:], w8[:, :, mrows],
                                x8[:, :, qt * NB:(qt + 1) * NB],
                                start=True, stop=True, perf_mode=DR)
                            nc.scalar.activation(
                                out=pT_bf[:, mc, qt * NB:(qt + 1) * NB],
                                in_=s[:],
                                func=mybir.ActivationFunctionType.Exp,
                                scale=LN2 / 128.0, bias=b_act[:, mc, :])
                        # running denominator (bf16 accumulator on DVE)
                        src = pT_bf[:, mc, :]
                        if mc == 0:
                            nc.vector.tensor_copy(out=dacc[:], in_=src)
                        else:
                            nc.vector.tensor_add(dacc[:], dacc[:], src)
                        # ctx-left accumulates chunk by chunk (bf16)
                        for cc in range(2):
                            for qt in range(2):
                                nc.tensor.matmul(
                                    ctxL[(qt, cc)][:],
                                    vT_bf[:, mc, cc * P:(cc + 1) * P],
                                    pT_bf[:, mc, qt * NB:(qt + 1) * NB],
                                    start=(mc == 0), stop=(mc == NMC - 1))

                # ---- phase 3: denom matmuls + ctx-right + epilogue ----
                with tc.tile_pool(name="psDR", bufs=1,
                                  space=bass.MemorySpace.PSUM) as psDR:
                    def ds_recip(ln):
                        cols = slice(ln * NB, (ln + 1) * NB)
                        ds = psDR.tile([P, NB], F32, tag="ds", name="ds", bufs=2)
                        nc.tensor.matmul(ds[:], ones_bf[:, 0:P], dacc[:, cols],
                                         start=True, stop=True)
                        nc.vector.reciprocal_approx_fast(out=recip[:, cols],
                                                         in_=ds[:])
                        return ds

                    def ctx_mms(ct, qt, cc):
                        for mc in range(NMC):
                            nc.tensor.matmul(
                                ct[:],
                                vT_bf[:, mc, cc * P:(cc + 1) * P],
                                pT_bf[:, mc, qt * NB:(qt + 1) * NB],
                                start=(mc == 0), stop=(mc == NMC - 1))

                    def ct_evict(ct, qt, cc, nsub, qpick):
                        rows = slice(cc * P, (cc + 1) * P)
                        sub = NB // nsub
                        for si in range(nsub):
                            c0 = qt * NB + si * sub
                            cols = slice(c0, c0 + sub)
                            pcols = slice(si * sub, (si + 1) * sub)
                            t = evict.tile([P, sub], F32, tag="t", name="t")
                            nc.vector.tensor_mul(t[:], ct[:, pcols],
                                                 recip[:, cols])
                            o = evict.tile([P, sub], BF16, tag="o", name="o")
                            nc.gpsimd.tensor_add(o[:], t[:], xr[cc][:, cols])
                            deng = (nc.sync, nc.scalar,
                                    nc.gpsimd)[(qpick + si) % 3]
                            deng.dma_start(out=out_d[rows, cols], in_=o[:])

                    # ds matmuls interleave between ctx-right tiles so the PE
                    # never head-blocks on the reciprocal chain
                    ds0 = ds_recip(0)
                    ds_recip(1)
                    # residual prep, pinned behind the denominator so the
                    # scheduler cannot hoist it into the scores-phase DVE queue
                    nc.vector.tensor_scalar(out=bv_late[:], in0=bv_sb[:],
                                            scalar1=ds0[:, 0:1],
                                            scalar2=ds0[:, 0:1],
                                            op0=mybir.AluOpType.add,
                                            op1=mybir.AluOpType.subtract)
                    for cc in range(2):
                        nc.vector.tensor_scalar_add(out=xr[cc][:],
                                                    in0=xbf[cc][:],
                                                    scalar1=bv_late[:, cc, :])
                    ctxR = {}
                    for k, (qt, cc) in enumerate(((2, 0), (2, 1), (3, 0), (3, 1))):
                        ct = psDR.tile([P, NB], F32, tag="cr", name="cr", bufs=2)
                        ctxR[(qt, cc)] = ct
                        ctx_mms(ct, qt, cc)
                        if k == 0:
                            ds_recip(2)
                            ds_recip(3)
                            ct_evict(ctxL[(0, 0)], 0, 0, 1, 0)
                            ct_evict(ctxL[(0, 1)], 0, 1, 1, 1)
                        elif k == 1:
                            ct_evict(ctxL[(1, 0)], 1, 0, 1, 2)
                            ct_evict(ctxL[(1, 1)], 1, 1, 1, 0)
                            ct_evict(ctxR[(2, 0)], 2, 0, 1, 1)
                            ct_evict(ctxR[(2, 1)], 2, 1, 1, 2)
                    ct_evict(ctxR[(3, 0)], 3, 0, 2, 0)
                    ct_evict(ctxR[(3, 1)], 3, 1, 4, 2)

    nc.compile()
    return nc


def get_compiled():
    global _COMPILED
    if _COMPILED is None:
        _COMPILED = build_nc()
    return _COMPILED


def make_in_maps(inputs):
    f8 = ml_dtypes.float8_e4m3
    x = np.ascontiguousarray(np.asarray(inputs["x"], dtype=np.float32))
    Wq = np.asarray(inputs["Wq"], np.float32)
    Wk = np.asarray(inputs["Wk"], np.float32)
    Wv = np.asarray(inputs["Wv"], np.float32)
    bq = np.asarray(inputs["bq"], np.float32)
    M = Wq.T @ Wk                               # scores_raw = x^T M x
    u = SCALE * (Wk.T @ bq)                     # per-key score bias u.x
    wvu = np.zeros((C, 272), np.float32)
    wvu[:, 0:C] = Wv.T
    wvu[:, C] = u
    shared = {
        "mt8": np.ascontiguousarray(KAPPA * M.T).astype(f8),
        "wvu8": wvu.astype(f8),
        "bv": np.asarray(inputs["bv"], np.float32).reshape(C, 1),
    }
    return [{"x8": x[i].astype(f8), "xbf": x[i].astype(ml_dtypes.bfloat16),
             **shared} for i in range(B)]


def run(inputs, trace=False, **kwargs):
    nc = get_compiled()
    res = run_bass_kernel_spmd(nc, make_in_maps(inputs),
                               core_ids=list(range(B)), trace=trace, **kwargs)
    out = np.stack([res.results[i]["out"] for i in range(B)], axis=0)
    return out.astype(np.float32), res


def kernel(**inputs):
    out, _ = run(inputs)
    return out


# revision 33
# speedup vs baseline: 1.1704x; 1.1704x over previous
"""Trainium2 Bass kernel for AttentionBlock (B=8, C=256, L=2048), data-parallel
over batch across 8 NeuronCores.

Math (one batch per core, x: [C, L]):
    t^T   = w8^T x8            w8 = fp8(kappa M x),  M = Wq^T Wk,  kappa = 128*SCALE/ln2
    pT    = exp(t*ln2/128 + ux - shift)   [m, l], m on partitions; the global
                               shift cancels in softmax
    denom = ones^T acc(pT)     (running bf16 accumulator on DVE)
    ctx   = vT^T pT            vT = x^T Wv^T in bf16; ux (per-key bq.Wk x)
                               rides along as a 257th output column of the
                               same projection
    out   = ctx * (1/denom) + (bf16(x) + bv)

The C=256 contractions (w projection, v projection, scores) run in fp8e4 with
perf_mode=DoubleRow: operands packed [128, 2, free], one instruction contracts
256 deep. On this silicon DoubleRow matches bf16 ALU throughput, so its win is
instruction/LDWEIGHTS count, and pT/vT stay bf16 (fp8 elementwise ops on DVE
run at 1x and dominate otherwise; measured).

Schedule:
  - fp32 x is never loaded; the residual uses bf16 x and the output is stored
    bf16 (error budget allows it: rel_err ~3.8e-3 vs the 2e-2 gate)
  - the v projection rides inside the scores loop (one DoubleRow matmul per
    key chunk) sharing the PSUM pool with the score tiles, so there is no
    pool-transition barrier before the scores start
  - context accumulation for the left half of the queries (qt 0,1) is
    interleaved into the scores phase chunk by chunk (PSUM: 4 banks scores/vp
    + 4 banks ctx-left); the right half runs after from the stored pT
  - exp runs on ACT (4 x 512-wide slices per chunk, ~2.7us) pacing the PE
    (~2.7us/chunk); the denominator accumulates on DVE in bf16 (2x mode)
  - the residual prep is pinned behind the denominator matmul via a dummy
    data dependency so the scheduler cannot hoist it into the scores-phase
    DVE stream (DVE executes strictly in order; one early op delays every
    later consumer)
  - ACT/DVE table loads and PE warmup happen during the initial DMA
"""

import math
import numpy as np
import ml_dtypes

import concourse.bass as bass
import concourse.tile as tile
from concourse import bacc, mybir
from concourse.bass_utils import run_bass_kernel_spmd

B, C, L = 8, 256, 2048
P = 128                 # partitions
NMC = L // P            # 16 m-chunks (key blocks)
NPAIR = NMC // 2
NB = 512                # matmul moving free dim
HALF = 1024
SCALE = float(C) ** -0.5
LN2 = math.log(2.0)
KAPPA = 128.0 * SCALE / LN2     # scores t = kappa * s_raw (baked into mt8 on host)
SHIFT = 2.0                     # global exp shift; cancels in softmax
WARMUP_MMS = 8

F32 = mybir.dt.float32
BF16 = mybir.dt.bfloat16
F8 = mybir.dt.float8e4
DR = mybir.MatmulPerfMode.DoubleRow

_COMPILED = None


def build_nc():
    nc = bacc.Bacc("TRN2", target_bir_lowering=False, debug=False, num_devices=8)

    x8_d = nc.dram_tensor("x8", [C, L], F8, kind="ExternalInput").ap()
    xbf_d = nc.dram_tensor("xbf", [C, L], BF16, kind="ExternalInput").ap()
    mt8_d = nc.dram_tensor("mt8", [C, C], F8, kind="ExternalInput").ap()
    wvu8_d = nc.dram_tensor("wvu8", [C, 272], F8, kind="ExternalInput").ap()
    bv_d = nc.dram_tensor("bv", [C, 1], F32, kind="ExternalInput").ap()
    out_d = nc.dram_tensor("out", [C, L], BF16, kind="ExternalOutput").ap()

    with tile.TileContext(nc) as tc:
        with (
            tc.tile_pool(name="const", bufs=1) as const,
            tc.tile_pool(name="data", bufs=1) as data,
            tc.tile_pool(name="evict", bufs=4) as evict,
        ):
            # ---- constants / warmup fodder ----
            ones_bf = const.tile([P, NB], BF16)
            nc.vector.memset(ones_bf[:], 1.0)
            ones8 = const.tile([P, 2, 16], F8)
            nc.gpsimd.memset(ones8[:], 1.0)
            tiny = const.tile([P, 2, 16], F32)

            x8 = data.tile([P, 2, L], F8, tag="x8", name="x8")
            xbf = [data.tile([P, L], BF16, tag=f"xbf{c}", name=f"xbf{c}")
                   for c in range(2)]
            mt8 = const.tile([P, 2, C], F8, tag="mt8")
            wvu8 = const.tile([P, 2, 272], F8, tag="wvu8")
            bv_sb = const.tile([P, 2, 1], F32, tag="bv")

            # first l-slice of x8 on several queues, then weights, then rest
            def x8_dma(c0, c1, eng):
                cols = slice(c0, c1)
                eng.dma_start(out=x8[:, :, cols],
                              in_=x8_d[:, cols].rearrange("(j p) l -> p j l",
                                                          p=P))

            # mt8 first (the w projection is the startup critical path),
            # then x8 slices in consumption order across all three queues
            nc.sync.dma_start(out=mt8[:],
                              in_=mt8_d.rearrange("(j p) o -> p j o", p=P))
            nc.scalar.dma_start(out=wvu8[:],
                                in_=wvu8_d.rearrange("(j p) o -> p j o", p=P))
            x8_dma(512, 1024, nc.gpsimd)
            x8_dma(1024, 1536, nc.scalar)
            x8_dma(0, 512, nc.sync)
            x8_dma(1536, 2048, nc.gpsimd)
            nc.scalar.dma_start(out=bv_sb[:],
                                in_=bv_d.rearrange("(j p) o -> p j o", p=P))

            w8 = data.tile([P, 2, L], F8, tag="w8", name="w8")
            vT_bf = data.tile([P, NMC, C], BF16, tag="vT")
            pT_bf = data.tile([P, NMC, L], BF16, tag="pT")
            b_act = data.tile([P, NMC, 1], F32, tag="b_act")
            bv_late = data.tile([P, 2, 1], F32, tag="bv_late")
            dacc = data.tile([P, L], BF16, tag="dacc")
            recip = data.tile([P, L], F32, tag="recip")
            xr = [data.tile([P, L], BF16, tag=f"xr{c}", name=f"xr{c}")
                  for c in range(2)]

            # ---- phase 1: PE warmup + w projection ----
            with tc.tile_pool(name="psA", bufs=1, space=bass.MemorySpace.PSUM) as psA:
                # warm the activation tables (one-time ~2.7us DMAs) and the PE
                # HAM clock-gate while x streams in
                warm = psA.tile([P, HALF], F32, tag="wp", name="warm", bufs=2)
                nc.vector.memset(tiny[:, 0, :], 1.0)
                nc.scalar.activation(out=tiny[:, 1, :], in_=tiny[:, 0, :],
                                     func=mybir.ActivationFunctionType.Exp,
                                     scale=1.0)
                nc.vector.reciprocal_approx_fast(out=tiny[:, 1, :],
                                                 in_=tiny[:, 0, :])
                for i in range(WARMUP_MMS):
                    nc.tensor.matmul(warm[:, 0:NB], ones_bf[:, 0:P],
                                     ones_bf[:], start=True, stop=True)
                nc.tensor.matmul(warm[0:16, 0:16], ones8[:], ones8[:],
                                 start=True, stop=True, perf_mode=DR)

                # w = kappa M x  (kappa baked into mt8 on host); one DoubleRow
                # matmul contracts the full 256 channels. The first two v/ux
                # projections sit between the halves so chunk 0/1's exp biases
                # are ready before the scores pipeline starts.
                def w_half(h):
                    hcols = slice(h * HALF, (h + 1) * HALF)
                    for oc in range(2):
                        wp = psA.tile([P, HALF], F32, tag="wp", name="wp",
                                      bufs=2)
                        for ln in range(2):
                            c0 = h * HALF + ln * NB
                            nc.tensor.matmul(
                                wp[:, ln * NB:(ln + 1) * NB],
                                mt8[:, :, oc * P:(oc + 1) * P],
                                x8[:, :, c0:c0 + NB],
                                start=True, stop=True, perf_mode=DR)
                        nc.vector.tensor_copy(out=w8[:, oc, hcols], in_=wp[:])

                def vp_chunk(pool, mc, bufs):
                    vp = pool.tile([P, 272], F32, tag="vp", name="vp",
                                   bufs=bufs)
                    nc.tensor.matmul(
                        vp[:], x8[:, :, mc * P:(mc + 1) * P], wvu8[:],
                        start=True, stop=True, perf_mode=DR)
                    nc.vector.tensor_copy(out=vT_bf[:, mc, :], in_=vp[:, 0:C])
                    nc.vector.tensor_scalar_add(out=b_act[:, mc, :],
                                                in0=vp[:, C:C + 1],
                                                scalar1=-SHIFT)

                w_half(0)
                vp_chunk(psA, 0, 2)
                vp_chunk(psA, 1, 2)
                w_half(1)

            # xbf for the residual - only needed by the epilogue; these queues
            # are idle during the scores phase
            nc.sync.dma_start(out=xbf[0][:], in_=xbf_d[0:P, :])
            nc.gpsimd.dma_start(out=xbf[1][:], in_=xbf_d[P:C, :])

            # ---- phase 2: v-projection + scores + exp + denom + ctx-left ----
            with tc.tile_pool(name="psCL", bufs=1,
                              space=bass.MemorySpace.PSUM) as psCL:
                ctxL = {(qt, cc): psCL.tile([P, NB], F32, tag=f"cl{qt}{cc}",
                                            name=f"cl{qt}{cc}", bufs=1)
                        for qt in range(2) for cc in range(2)}

                with tc.tile_pool(name="psB", bufs=1,
                                  space=bass.MemorySpace.PSUM) as psB:
                    for mc in range(NMC):
                        mrows = slice(mc * P, (mc + 1) * P)
                        # v/ux projection (chunks 0/1 were done in phase 1);
                        # chunk mc+2's projection is issued here so it stays
                        # two chunks ahead of its ctx-left consumer
                        if mc + 2 < NMC:
                            vp_chunk(psB, mc + 2, 1)
                        # scores + exp, 512 columns at a time
                        for qt in range(4):
                            s = psB.tile([P, NB], F32, tag="s", name="s",
                                         bufs=3)
                            nc.tensor.matmul(
                                s[:], w8[:, :, mrows],
                                x8[:, :, qt * NB:(qt + 1) * NB],
                                start=True, stop=True, perf_mode=DR)
                            nc.scalar.activation(
                                out=pT_bf[:, mc, qt * NB:(qt + 1) * NB],
                                in_=s[:],
                                func=mybir.ActivationFunctionType.Exp,
                                scale=LN2 / 128.0, bias=b_act[:, mc, :])
                        # running denominator (bf16 accumulator on DVE)
                        src = pT_bf[:, mc, :]
                        if mc == 0:
                            nc.vector.tensor_copy(out=dacc[:], in_=src)
                        else:
                            nc.vector.tensor_add(dacc[:], dacc[:], src)
                        # ctx-left accumulates chunk by chunk (bf16)
                        for cc in range(2):
                            for qt in range(2):
                                nc.tensor.matmul(
                                    ctxL[(qt, cc)][:],
                                    vT_bf[:, mc, cc * P:(cc + 1) * P],
                                    pT_bf[:, mc, qt * NB:(qt + 1) * NB],
                                    start=(mc == 0), stop=(mc == NMC - 1))

                # ---- phase 3: denom matmuls + ctx-right + epilogue ----
                with tc.tile_pool(name="psDR", bufs=1,
                                  space=bass.MemorySpace.PSUM) as psDR:
                    def ds_recip(ln):
                        cols = slice(ln * NB, (ln + 1) * NB)
                        ds = psDR.tile([P, NB], F32, tag="ds", name="ds", bufs=2)
                        nc.tensor.matmul(ds[:], ones_bf[:, 0:P], dacc[:, cols],
                                         start=True, stop=True)
                        nc.vector.reciprocal_approx_fast(out=recip[:, cols],
                                                         in_=ds[:])
                        return ds

                    def ctx_mms(ct, qt, cc):
                        for mc in range(NMC):
                            nc.tensor.matmul(
                                ct[:],
                                vT_bf[:, mc, cc * P:(cc + 1) * P],
                                pT_bf[:, mc, qt * NB:(qt + 1) * NB],
                                start=(mc == 0), stop=(mc == NMC - 1))

                    def ct_evict(ct, qt, cc, nsub, qpick):
                        rows = slice(cc * P, (cc + 1) * P)
                        sub = NB // nsub
                        for si in range(nsub):
                            c0 = qt * NB + si * sub
                            cols = slice(c0, c0 + sub)
                            pcols = slice(si * sub, (si + 1) * sub)
                            t = evict.tile([P, sub], F32, tag="t", name="t")
                            nc.vector.tensor_mul(t[:], ct[:, pcols],
                                                 recip[:, cols])
                            o = evict.tile([P, sub], BF16, tag="o", name="o")
                            nc.gpsimd.tensor_add(o[:], t[:], xr[cc][:, cols])
                            deng = (nc.sync, nc.scalar,
                                    nc.gpsimd)[(qpick + si) % 3]
                            deng.dma_start(out=out_d[rows, cols], in_=o[:])

                    # ds matmuls interleave between ctx-right tiles so the PE
                    # never head-blocks on the reciprocal chain
                    ds0 = ds_recip(0)
                    ds_recip(1)
                    # residual prep, pinned behind the denominator so the
                    # scheduler cannot hoist it into the scores-phase DVE queue
                    nc.vector.tensor_scalar(out=bv_late[:], in0=bv_sb[:],
                                            scalar1=ds0[:, 0:1],
                                            scalar2=ds0[:, 0:1],
                                            op0=mybir.AluOpType.add,
                                            op1=mybir.AluOpType.subtract)
                    for cc in range(2):
                        nc.vector.tensor_scalar_add(out=xr[cc][:],
                                                    in0=xbf[cc][:],
                                                    scalar1=bv_late[:, cc, :])
                    ctxR = {}
                    for k, (qt, cc) in enumerate(((2, 0), (2, 1), (3, 0), (3, 1))):
                        ct = psDR.tile([P, NB], F32, tag="cr", name="cr", bufs=2)
                        ctxR[(qt, cc)] = ct
                        ctx_mms(ct, qt, cc)
                        if k == 0:
                            ds_recip(2)
                            ds_recip(3)
                            ct_evict(ctxL[(0, 0)], 0, 0, 1, 0)
                            ct_evict(ctxL[(0, 1)], 0, 1, 1, 1)
                        elif k == 1:
                            ct_evict(ctxL[(1, 0)], 1, 0, 1, 2)
                            ct_evict(ctxL[(1, 1)], 1, 1, 1, 0)
                            ct_evict(ctxR[(2, 0)], 2, 0, 1, 1)
                            ct_evict(ctxR[(2, 1)], 2, 1, 1, 2)
                    ct_evict(ctxR[(3, 0)], 3, 0, 2, 0)
                    ct_evict(ctxR[(3, 1)], 3, 1, 4, 2)

    nc.compile()
    return nc


def get_compiled():
    global _COMPILED
    if _COMPILED is None:
        _COMPILED = build_nc()
    return _COMPILED


def make_in_maps(inputs):
    f8 = ml_dtypes.float8_e4m3
    x = np.ascontiguousarray(np.asarray(inputs["x"], dtype=np.float32))
    Wq = np.asarray(inputs["Wq"], np.float32)
    Wk = np.asarray(inputs["Wk"], np.float32)
    Wv = np.asarray(inputs["Wv"], np.float32)
    bq = np.asarray(inputs["bq"], np.float32)
    M = Wq.T @ Wk                               # scores_raw = x^T M x
    u = SCALE * (Wk.T @ bq)                     # per-key score bias u.x
    wvu = np.zeros((C, 272), np.float32)
    wvu[:, 0:C] = Wv.T
    wvu[:, C] = u
    shared = {
        "mt8": np.ascontiguousarray(KAPPA * M.T).astype(f8),
        "wvu8": wvu.astype(f8),
        "bv": np.asarray(inputs["bv"], np.float32).reshape(C, 1),
    }
    return [{"x8": x[i].astype(f8), "xbf": x[i].astype(ml_dtypes.bfloat16),
             **shared} for i in range(B)]


def run(inputs, trace=False, **kwargs):
    nc = get_compiled()
    res = run_bass_kernel_spmd(nc, make_in_maps(inputs),
                               core_ids=list(range(B)), trace=trace, **kwargs)
    out = np.stack([res.results[i]["out"] for i in range(B)], axis=0)
    return out.astype(np.float32), res


def kernel(**inputs):
    out, _ = run(inputs)
    return out


# revision 34
# speedup vs baseline: 1.1823x; 1.0102x over previous
"""Trainium2 Bass kernel for AttentionBlock (B=8, C=256, L=2048), data-parallel
over batch across 8 NeuronCores.

Math (one batch per core, x: [C, L]):
    t^T   = w8^T x8            w8 = fp8(kappa M x),  M = Wq^T Wk,  kappa = 128*SCALE/ln2
    pT    = exp(t*ln2/128 + ux - shift)   [m, l], m on partitions; the global
                               shift cancels in softmax
    denom = ones^T acc(pT)     (running bf16 accumulator on DVE)
    ctx   = vT^T pT            vT = x^T Wv^T in bf16; ux (per-key bq.Wk x)
                               rides along as a 257th output column of the
                               same projection
    out   = ctx * (1/denom) + (bf16(x) + bv)

The C=256 contractions (w projection, v projection, scores) run in fp8e4 with
perf_mode=DoubleRow: operands packed [128, 2, free], one instruction contracts
256 deep. On this silicon DoubleRow matches bf16 ALU throughput, so its win is
instruction/LDWEIGHTS count, and pT/vT stay bf16 (fp8 elementwise ops on DVE
run at 1x and dominate otherwise; measured).

Schedule:
  - fp32 x is never loaded; the residual uses bf16 x and the output is stored
    bf16 (error budget allows it: rel_err ~3.8e-3 vs the 2e-2 gate)
  - the v projection rides inside the scores loop (one DoubleRow matmul per
    key chunk) sharing the PSUM pool with the score tiles, so there is no
    pool-transition barrier before the scores start
  - context accumulation for the left half of the queries (qt 0,1) is
    interleaved into the scores phase chunk by chunk (PSUM: 4 banks scores/vp
    + 4 banks ctx-left); the right half runs after from the stored pT
  - exp runs on ACT (4 x 512-wide slices per chunk, ~2.7us) pacing the PE
    (~2.7us/chunk); the denominator accumulates on DVE in bf16 (2x mode)
  - the residual prep is pinned behind the denominator matmul via a dummy
    data dependency so the scheduler cannot hoist it into the scores-phase
    DVE stream (DVE executes strictly in order; one early op delays every
    later consumer)
  - ACT/DVE table loads and PE warmup happen during the initial DMA
"""

import math
import numpy as np
import ml_dtypes

import concourse.bass as bass
import concourse.tile as tile
from concourse import bacc, mybir
from concourse.bass_utils import run_bass_kernel_spmd

B, C, L = 8, 256, 2048
P = 128                 # partitions
NMC = L // P            # 16 m-chunks (key blocks)
NPAIR = NMC // 2
NB = 512                # matmul moving free dim
HALF = 1024
SCALE = float(C) ** -0.5
LN2 = math.log(2.0)
KAPPA = 128.0 * SCALE / LN2     # scores t = kappa * s_raw (baked into mt8 on host)
SHIFT = 2.0                     # global exp shift; cancels in softmax
WARMUP_MMS = 8

F32 = mybir.dt.float32
BF16 = mybir.dt.bfloat16
F8 = mybir.dt.float8e4
DR = mybir.MatmulPerfMode.DoubleRow

_COMPILED = None


def build_nc():
    nc = bacc.Bacc("TRN2", target_bir_lowering=False, debug=False, num_devices=8)

    x8_d = nc.dram_tensor("x8", [C, L], F8, kind="ExternalInput").ap()
    xbf_d = nc.dram_tensor("xbf", [C, L], BF16, kind="ExternalInput").ap()
    mt8_d = nc.dram_tensor("mt8", [C, C], F8, kind="ExternalInput").ap()
    wvu8_d = nc.dram_tensor("wvu8", [C, 272], F8, kind="ExternalInput").ap()
    bv_d = nc.dram_tensor("bv", [C, 1], F32, kind="ExternalInput").ap()
    out_d = nc.dram_tensor("out", [C, L], BF16, kind="ExternalOutput").ap()

    with tile.TileContext(nc) as tc:
        with (
            tc.tile_pool(name="const", bufs=1) as const,
            tc.tile_pool(name="data", bufs=1) as data,
            tc.tile_pool(name="evict", bufs=4) as evict,
        ):
            # ---- constants / warmup fodder ----
            ones_bf = const.tile([P, NB], BF16)
            nc.vector.memset(ones_bf[:], 1.0)
            ones8 = const.tile([P, 2, 16], F8)
            nc.gpsimd.memset(ones8[:], 1.0)
            tiny = const.tile([P, 2, 16], F32)

            x8 = data.tile([P, 2, L], F8, tag="x8", name="x8")
            xbf = [data.tile([P, L], BF16, tag=f"xbf{c}", name=f"xbf{c}")
                   for c in range(2)]
            mt8 = const.tile([P, 2, C], F8, tag="mt8")
            wvu8 = const.tile([P, 2, 272], F8, tag="wvu8")
            bv_sb = const.tile([P, 2, 1], F32, tag="bv")

            # first l-slice of x8 on several queues, then weights, then rest
            def x8_dma(c0, c1, eng):
                cols = slice(c0, c1)
                eng.dma_start(out=x8[:, :, cols],
                              in_=x8_d[:, cols].rearrange("(j p) l -> p j l",
                                                          p=P))

            # mt8 first (the w projection is the startup critical path),
            # then x8 slices in consumption order across all three queues
            nc.sync.dma_start(out=mt8[:],
                              in_=mt8_d.rearrange("(j p) o -> p j o", p=P))
            x8_dma(512, 1024, nc.gpsimd)
            x8_dma(1024, 1536, nc.scalar)
            x8_dma(0, 512, nc.sync)
            x8_dma(1536, 2048, nc.gpsimd)
            nc.scalar.dma_start(out=wvu8[:],
                                in_=wvu8_d.rearrange("(j p) o -> p j o", p=P))
            nc.scalar.dma_start(out=bv_sb[:],
                                in_=bv_d.rearrange("(j p) o -> p j o", p=P))

            w8 = data.tile([P, 2, L], F8, tag="w8", name="w8")
            vT_bf = data.tile([P, NMC, C], BF16, tag="vT")
            pT_bf = data.tile([P, NMC, L], BF16, tag="pT")
            b_act = data.tile([P, NMC, 1], F32, tag="b_act")
            bv_late = data.tile([P, 2, 1], F32, tag="bv_late")
            dacc = data.tile([P, L], BF16, tag="dacc")
            recip = data.tile([P, L], F32, tag="recip")
            xr = [data.tile([P, L], BF16, tag=f"xr{c}", name=f"xr{c}")
                  for c in range(2)]

            # ---- phase 1: PE warmup + w projection ----
            with tc.tile_pool(name="psA", bufs=1, space=bass.MemorySpace.PSUM) as psA:
                # warm the activation tables (one-time ~2.7us DMAs) and the PE
                # HAM clock-gate while x streams in
                warm = psA.tile([P, HALF], F32, tag="wp", name="warm", bufs=2)
                nc.vector.memset(tiny[:, 0, :], 1.0)
                nc.scalar.activation(out=tiny[:, 1, :], in_=tiny[:, 0, :],
                                     func=mybir.ActivationFunctionType.Exp,
                                     scale=1.0)
                nc.vector.reciprocal_approx_fast(out=tiny[:, 1, :],
                                                 in_=tiny[:, 0, :])
                for i in range(WARMUP_MMS):
                    nc.tensor.matmul(warm[:, 0:NB], ones_bf[:, 0:P],
                                     ones_bf[:], start=True, stop=True)
                nc.tensor.matmul(warm[0:16, 0:16], ones8[:], ones8[:],
                                 start=True, stop=True, perf_mode=DR)

                # w = kappa M x  (kappa baked into mt8 on host); one DoubleRow
                # matmul contracts the full 256 channels
                for h in range(2):
                    hcols = slice(h * HALF, (h + 1) * HALF)
                    for oc in range(2):
                        wp = psA.tile([P, HALF], F32, tag="wp", name="wp",
                                      bufs=2)
                        for ln in range(2):
                            c0 = h * HALF + ln * NB
                            nc.tensor.matmul(
                                wp[:, ln * NB:(ln + 1) * NB],
                                mt8[:, :, oc * P:(oc + 1) * P],
                                x8[:, :, c0:c0 + NB],
                                start=True, stop=True, perf_mode=DR)
                        nc.vector.tensor_copy(out=w8[:, oc, hcols], in_=wp[:])

            # xbf for the residual - only needed by the epilogue; these queues
            # are idle during the scores phase
            nc.sync.dma_start(out=xbf[0][:], in_=xbf_d[0:P, :])
            nc.gpsimd.dma_start(out=xbf[1][:], in_=xbf_d[P:C, :])

            # ---- phase 2: v-projection + scores + exp + denom + ctx-left ----
            with tc.tile_pool(name="psCL", bufs=1,
                              space=bass.MemorySpace.PSUM) as psCL:
                ctxL = {(qt, cc): psCL.tile([P, NB], F32, tag=f"cl{qt}{cc}",
                                            name=f"cl{qt}{cc}", bufs=1)
                        for qt in range(2) for cc in range(2)}

                with tc.tile_pool(name="psB", bufs=1,
                                  space=bass.MemorySpace.PSUM) as psB:
                    for mc in range(NMC):
                        mrows = slice(mc * P, (mc + 1) * P)
                        # v/ux projection for this key chunk
                        vp = psB.tile([P, 272], F32, tag="vp", name="vp", bufs=1)
                        nc.tensor.matmul(
                            vp[:], x8[:, :, mrows], wvu8[:],
                            start=True, stop=True, perf_mode=DR)
                        nc.vector.tensor_copy(out=vT_bf[:, mc, :],
                                              in_=vp[:, 0:C])
                        nc.vector.tensor_scalar_add(out=b_act[:, mc, :],
                                                    in0=vp[:, C:C + 1],
                                                    scalar1=-SHIFT)
                        # scores + exp, 512 columns at a time
                        for qt in range(4):
                            s = psB.tile([P, NB], F32, tag="s", name="s",
                                         bufs=3)
                            nc.tensor.matmul(
                                s[:], w8[:, :, mrows],
                                x8[:, :, qt * NB:(qt + 1) * NB],
                                start=True, stop=True, perf_mode=DR)
                            nc.scalar.activation(
                                out=pT_bf[:, mc, qt * NB:(qt + 1) * NB],
                                in_=s[:],
                                func=mybir.ActivationFunctionType.Exp,
                                scale=LN2 / 128.0, bias=b_act[:, mc, :])
                        # running denominator (bf16 accumulator on DVE)
                        src = pT_bf[:, mc, :]
                        if mc == 0:
                            nc.vector.tensor_copy(out=dacc[:], in_=src)
                        else:
                            nc.vector.tensor_add(dacc[:], dacc[:], src)
                        # ctx-left accumulates chunk by chunk (bf16)
                        for cc in range(2):
                            for qt in range(2):
                                nc.tensor.matmul(
                                    ctxL[(qt, cc)][:],
                                    vT_bf[:, mc, cc * P:(cc + 1) * P],
                                    pT_bf[:, mc, qt * NB:(qt + 1) * NB],
                                    start=(mc == 0), stop=(mc == NMC - 1))

                # ---- phase 3: denom matmuls + ctx-right + epilogue ----
                with tc.tile_pool(name="psDR", bufs=1,
                                  space=bass.MemorySpace.PSUM) as psDR:
                    def ds_recip(ln):
                        cols = slice(ln * NB, (ln + 1) * NB)
                        ds = psDR.tile([P, NB], F32, tag="ds", name="ds", bufs=2)
                        nc.tensor.matmul(ds[:], ones_bf[:, 0:P], dacc[:, cols],
                                         start=True, stop=True)
                        nc.vector.reciprocal_approx_fast(out=recip[:, cols],
                                                         in_=ds[:])
                        return ds

                    def ctx_mms(ct, qt, cc):
                        for mc in range(NMC):
                            nc.tensor.matmul(
                                ct[:],
                                vT_bf[:, mc, cc * P:(cc + 1) * P],
                                pT_bf[:, mc, qt * NB:(qt + 1) * NB],
                                start=(mc == 0), stop=(mc == NMC - 1))

                    def ct_evict(ct, qt, cc, nsub, qpick):
                        rows = slice(cc * P, (cc + 1) * P)
                        sub = NB // nsub
                        for si in range(nsub):
                            c0 = qt * NB + si * sub
                            cols = slice(c0, c0 + sub)
                            pcols = slice(si * sub, (si + 1) * sub)
                            t = evict.tile([P, sub], F32, tag="t", name="t")
                            nc.vector.tensor_mul(t[:], ct[:, pcols],
                                                 recip[:, cols])
                            o = evict.tile([P, sub], BF16, tag="o", name="o")
                            nc.gpsimd.tensor_add(o[:], t[:], xr[cc][:, cols])
                            deng = (nc.sync, nc.scalar,
                                    nc.gpsimd)[(qpick + si) % 3]
                            deng.dma_start(out=out_d[rows, cols], in_=o[:])

                    # ds matmuls interleave between ctx-right tiles so the PE
                    # never head-blocks on the reciprocal chain
                    ds0 = ds_recip(0)
                    ds_recip(1)
                    # residual prep, pinned behind the denominator so the
                    # scheduler cannot hoist it into the scores-phase DVE queue
                    nc.vector.tensor_scalar(out=bv_late[:], in0=bv_sb[:],
                                            scalar1=ds0[:, 0:1],
                                            scalar2=ds0[:, 0:1],
                                            op0=mybir.AluOpType.add,
                                            op1=mybir.AluOpType.subtract)
                    for cc in range(2):
                        nc.vector.tensor_scalar_add(out=xr[cc][:],
                                                    in0=xbf[cc][:],
                                                    scalar1=bv_late[:, cc, :])
                    ctxR = {}
                    for k, (qt, cc) in enumerate(((2, 0), (2, 1), (3, 0), (3, 1))):
                        ct = psDR.tile([P, NB], F32, tag="cr", name="cr", bufs=2)
                        ctxR[(qt, cc)] = ct
                        ctx_mms(ct, qt, cc)
                        if k == 0:
                            ds_recip(2)
                            ds_recip(3)
                            ct_evict(ctxL[(0, 0)], 0, 0, 1, 0)
                            ct_evict(ctxL[(0, 1)], 0, 1, 1, 1)
                        elif k == 1:
                            ct_evict(ctxL[(1, 0)], 1, 0, 1, 2)
                            ct_evict(ctxL[(1, 1)], 1, 1, 1, 0)
                            ct_evict(ctxR[(2, 0)], 2, 0, 1, 1)
                            ct_evict(ctxR[(2, 1)], 2, 1, 1, 2)
                    ct_evict(ctxR[(3, 0)], 3, 0, 2, 0)
                    ct_evict(ctxR[(3, 1)], 3, 1, 4, 2)

    nc.compile()
    return nc


def get_compiled():
    global _COMPILED
    if _COMPILED is None:
        _COMPILED = build_nc()
    return _COMPILED


def make_in_maps(inputs):
    f8 = ml_dtypes.float8_e4m3
    x = np.ascontiguousarray(np.asarray(inputs["x"], dtype=np.float32))
    Wq = np.asarray(inputs["Wq"], np.float32)
    Wk = np.asarray(inputs["Wk"], np.float32)
    Wv = np.asarray(inputs["Wv"], np.float32)
    bq = np.asarray(inputs["bq"], np.float32)
    M = Wq.T @ Wk                               # scores_raw = x^T M x
    u = SCALE * (Wk.T @ bq)                     # per-key score bias u.x
    wvu = np.zeros((C, 272), np.float32)
    wvu[:, 0:C] = Wv.T
    wvu[:, C] = u
    shared = {
        "mt8": np.ascontiguousarray(KAPPA * M.T).astype(f8),
        "wvu8": wvu.astype(f8),
        "bv": np.asarray(inputs["bv"], np.float32).reshape(C, 1),
    }
    return [{"x8": x[i].astype(f8), "xbf": x[i].astype(ml_dtypes.bfloat16),
             **shared} for i in range(B)]


def run(inputs, trace=False, **kwargs):
    nc = get_compiled()
    res = run_bass_kernel_spmd(nc, make_in_maps(inputs),
                               core_ids=list(range(B)), trace=trace, **kwargs)
    out = np.stack([res.results[i]["out"] for i in range(B)], axis=0)
    return out.astype(np.float32), res


def kernel(**inputs):
    out, _ = run(inputs)
    return out
